# revision 1
# baseline (speedup 1.0000x reference)
"""Trainium2 Bass kernel for the chain-DAG generator MLP.

Math (per batch row b, node i in topological order 0..15):
    c_i = input_c @ Wc[:, 16i:16i+16]
    d_i = input_d @ theta[:, 16i:16i+16],  theta = mu + softplus(sigma)*noise_d
    h_i = relu(c_i @ W1c_i + d_i @ W1d_i + n_i @ W1n_i + p_i * w_p_i + b1_i)
    out_i = h_i @ W2_i + b2_i,   p_i = out_{i-1} for i in 1..13 (0,14,15 roots)

Device mapping (data-parallel over batch on 8 cores, B_s=16384 rows/core,
32 chunks of 512 batch columns, hidden-on-partition layout):
  - base per node-pair q: ONE K=48 fp32r matmul over [input_c^T; input_d^T;
    ones; pad; noise_2q^T; noise_2q+1^T] with folded weights (Wc/theta
    absorbed into the lhsT, b1 via the ones row) into PSUM bank [128, 512].
  - chain: relu(h_i) into a per-node SBUF tile at partitions 0:64;
    child_pre += outer(W2_i, w_p_{i+1})^T @ h_i — one K=64 matmul into the
    child's bank (M padded to 128; the parent scalar never materializes;
    b2_i folded into the child's bias row).
  - collect: per-node K=64 matmul (K=128 for the 12/13 and 14/15 tiles)
    accumulating W2-contractions into an output bank; rows 0:16 + b2 move
    to SBUF; DMA out transposed [16, B_s]; host transposes back.

HW constraint honored throughout: every matmul in one PSUM accumulation
group uses the same tile config — size (64,128) at position (0,0) — since
mixed tile positions inside a group corrupt execution, and fp32r cannot
column-tile (so M is always padded to 128 with zero weight columns).
"""

import threading

import numpy as np

import concourse.bacc as bacc
import concourse.mybir as mybir
from concourse.bass_utils import run_bass_kernel_spmd
from concourse.tile import TileContext

N_CORES = 8
B_FULL = 131072
B_S = B_FULL // N_CORES  # 16384
CHUNK = 512
I_DIM = 16
N_PAIRS = 8

F32 = mybir.dt.float32
FR = mybir.dt.float32r


def build_nc(b_s: int = B_S, chunk: int = CHUNK):
    """Build the single-core program (SPMD: same program on all cores)."""
    assert b_s % chunk == 0
    n_chunks = b_s // chunk

    nc = bacc.Bacc(
        "TRN2", target_bir_lowering=False, debug=False, num_devices=N_CORES
    )

    # Per-core inputs
    s_d = nc.dram_tensor("S", [16, b_s], FR, kind="ExternalInput").ap()
    nt_d = nc.dram_tensor("NT", [256, b_s], FR, kind="ExternalInput").ap()
    # Folded weights (replicated on every core)
    px_d = nc.dram_tensor("PX", [48, 128 * N_PAIRS], FR, kind="ExternalInput").ap()
    mc_d = nc.dram_tensor("MC", [64, 128 * 13], FR, kind="ExternalInput").ap()
    cl_d = nc.dram_tensor("CLW", [64, 128 * 16], FR, kind="ExternalInput").ap()
    b2_d = nc.dram_tensor("B2", [16, 1], F32, kind="ExternalInput").ap()
    out_d = nc.dram_tensor("OUT", [16, b_s], F32, kind="ExternalOutput").ap()

    with TileContext(nc) as tc:
        with (
            tc.tile_pool(name="consts", bufs=1) as cpool,
            tc.tile_pool(name="ins", bufs=20) as ipool,
            tc.tile_pool(name="hbuf", bufs=18) as hpool,
            tc.tile_pool(name="obuf", bufs=3) as opool,
            tc.tile_pool(name="pairs", bufs=6, space="PSUM") as ppool,
            tc.tile_pool(name="outp", bufs=2, space="PSUM") as qpool,
        ):
            px_t = cpool.tile([48, 128 * N_PAIRS], FR)
            nc.sync.dma_start(out=px_t[:, :], in_=px_d[:, :])
            mc_t = cpool.tile([64, 128 * 13], FR)
            nc.sync.dma_start(out=mc_t[:, :], in_=mc_d[:, :])
            cl_t = cpool.tile([64, 128 * 16], FR)
            nc.sync.dma_start(out=cl_t[:, :], in_=cl_d[:, :])
            b2_t = cpool.tile([16, 1], F32)
            nc.sync.dma_start(out=b2_t[:, :], in_=b2_d[:, :])

            for ch in range(n_chunks):
                c0 = ch * chunk
                sl = slice(c0, c0 + chunk)

                # --- base: one K=48 matmul per pair bank ---
                banks = []
                for q in range(N_PAIRS):
                    x_q = ipool.tile([48, chunk], FR, tag="x", name=f"x_{ch}_{q}")
                    nc.sync.dma_start(out=x_q[0:16, :], in_=s_d[:, sl])
                    nc.sync.dma_start(
                        out=x_q[16:48, :], in_=nt_d[32 * q : 32 * q + 32, sl]
                    )
                    bank = ppool.tile(
                        [128, chunk], F32, tag="bank", name=f"bank_{ch}_{q}"
                    )
                    banks.append(bank)
                    nc.tensor.matmul(
                        out=bank[:, :],
                        lhsT=px_t[:, 128 * q : 128 * (q + 1)],
                        rhs=x_q[:, :],
                        start=True,
                        stop=(q == 7),  # bank 7 takes no chain matmul
                        skip_group_check=True,
                    )

                # --- chain + collect ---
                # h tiles: one [64, chunk] per node, always at partitions
                # 0:64 so every chain/collect matmul runs at tile config
                # (64, 128) @ (0, 0).
                bank_out = qpool.tile([128, chunk], F32, tag="bout")
                for i in range(I_DIM):
                    q, r = divmod(i, 2)
                    h = hpool.tile([64, chunk], FR, tag="h", name=f"h_{ch}_{i}")
                    brows = slice(64 * r, 64 * (r + 1))
                    if i % 2 == 0:
                        nc.scalar.activation(
                            h[:, :],
                            banks[q][brows, :],
                            mybir.ActivationFunctionType.Relu,
                        )
                    else:
                        nc.vector.tensor_scalar_max(
                            out=h[:, :], in0=banks[q][brows, :], scalar1=0.0
                        )
                    if i <= 12:
                        rc = (i + 1) % 2
                        nc.tensor.matmul(
                            out=banks[(i + 1) // 2][:, :],
                            lhsT=mc_t[:, 128 * i : 128 * (i + 1)],
                            rhs=h[:, :],
                            start=False,
                            stop=(rc == 1),  # chain(2q) closes pair q's group
                            skip_group_check=True,
                        )
                    nc.tensor.matmul(
                        out=bank_out[:, :],
                        lhsT=cl_t[:, 128 * i : 128 * (i + 1)],
                        rhs=h[:, :],
                        start=(i == 0),
                        stop=(i == 15),
                        skip_group_check=True,
                    )

                o_t = opool.tile([16, chunk], F32, tag="o")
                nc.vector.tensor_scalar_add(
                    out=o_t[:, :], in0=bank_out[0:16, :], scalar1=b2_t[:, 0:1]
                )
                nc.sync.dma_start(out=out_d[:, sl], in_=o_t[:, :])

    nc.compile()
    return nc


def prep_weights(noise_d, mu, sigma, Wc, W1, b1, W2, b2):
    """Fold the tiny parameter tensors into the device weight layout."""
    theta = mu + np.log1p(np.exp(sigma)) * noise_d  # [4, 256]
    w_p = W1[:, 48, :]  # [16, 64]
    b1e = b1.copy()  # [16, 64]
    for i in range(1, 14):  # nodes with parent i-1
        b1e[i] = b1[i] + w_p[i] * b2[i - 1]

    # base lhsT per pair: rows [A_c(10); A_d(4); b1e(1); 0(1); A_n block-diag(32)]
    px = np.zeros((48, 128 * N_PAIRS), np.float32)
    for q in range(N_PAIRS):
        for r in range(2):
            i = 2 * q + r
            cols = slice(128 * q + 64 * r, 128 * q + 64 * (r + 1))
            px[0:10, cols] = Wc[:, 16 * i : 16 * (i + 1)] @ W1[i, 0:16, :]
            px[10:14, cols] = theta[:, 16 * i : 16 * (i + 1)] @ W1[i, 16:32, :]
            px[14, cols] = b1e[i]
            px[16 + 16 * r : 32 + 16 * r, cols] = W1[i, 32:48, :]

    # chain lhsT for node i -> child i+1 (child's rows at 64*((i+1)%2))
    mc = np.zeros((64, 128 * 13), np.float32)
    for i in range(13):
        c0 = 128 * i + 64 * ((i + 1) % 2)
        mc[:, c0 : c0 + 64] = np.outer(W2[i], w_p[i + 1])

    # collect lhsT: one [64, 128] block per node; real column = node id
    cl = np.zeros((64, 128 * 16), np.float32)
    for i in range(16):
        cl[:, 128 * i + i] = W2[i]

    return {
        "PX": px,
        "MC": mc,
        "CLW": cl,
        "B2": b2.reshape(16, 1).astype(np.float32),
    }


def prep_core_inputs(noise, input_c, input_d, c):
    """Shard + transpose per-core batch inputs."""
    b0, b1_ = c * B_S, (c + 1) * B_S
    s = np.zeros((16, B_S), np.float32)
    s[0:10] = input_c[b0:b1_].T
    s[10:14] = input_d[b0:b1_].T
    s[14] = 1.0
    nt = np.ascontiguousarray(noise[b0:b1_].T)
    return {"S": s, "NT": nt}


_NC_LOCK = threading.Lock()
_NC_CACHE = {}


def _get_nc():
    with _NC_LOCK:
        if "nc" not in _NC_CACHE:
            _NC_CACHE["nc"] = build_nc()
        return _NC_CACHE["nc"]


def kernel(noise, input_c, input_d, noise_d, mu, sigma, Wc, W1, b1, W2, b2):
    noise = np.asarray(noise, np.float32)
    input_c = np.asarray(input_c, np.float32)
    input_d = np.asarray(input_d, np.float32)
    w = prep_weights(
        np.asarray(noise_d, np.float32),
        np.asarray(mu, np.float32),
        np.asarray(sigma, np.float32),
        np.asarray(Wc, np.float32),
        np.asarray(W1, np.float32),
        np.asarray(b1, np.float32),
        np.asarray(W2, np.float32),
        np.asarray(b2, np.float32),
    )
    in_maps = []
    for c in range(N_CORES):
        m = prep_core_inputs(noise, input_c, input_d, c)
        m.update(w)
        in_maps.append(m)

    nc = _get_nc()
    res = run_bass_kernel_spmd(nc, in_maps, list(range(N_CORES)))
    out = np.concatenate(
        [res.results[c]["OUT"].T for c in range(N_CORES)], axis=0
    )
    return np.ascontiguousarray(out, np.float32)

